# revision 20
# baseline (speedup 1.0000x reference)
"""GNN message-passing layer on 8 Trainium2 NeuronCores.

Reference computation:
    proj = relu(h @ W.T)              # [N, 128]
    out  = segment_sum(proj[src], dst, N)

Strategy (edge-parallel, dst-partitioned):
  * Output nodes are partitioned contiguously across the 8 cores
    (12500 nodes/core); each core receives exactly the edges whose dst
    it owns (~100k edges/core).
  * Per core, owned nodes are sorted by in-degree (descending) and
    edges are organized into "rounds": round k holds the k-th incoming
    edge of every node that has more than k edges.  Within a round each
    active node appears exactly once, at a slot equal to its position
    in the degree-sorted order - so round k's messages accumulate into
    accumulator columns [0, cnt_k) with plain element-wise adds; no
    scatter is ever needed on-device.
  * Source features are fetched per-edge with the GPSIMD dma_gather
    custom instruction in transposed mode, which lands features on
    partitions - directly consumable as the moving operand of a
    matmul.  Rows are stored hi||lo (bf16 split of the fp32 value,
    512B/row - measured free vs 256B on HW); three bf16 matmuls
    accumulate h_hi@W_hi + h_lo@W_hi + h_hi@W_lo in fp32 PSUM (~1e-6
    relative).  Gathers are spread round-robin over 4 SWDGE queues
    with a 16-buffer pipeline: the per-call ucode overhead (~1.2us,
    num_idxs hard-capped at 512) is the bottleneck, and 4 queues
    process calls concurrently (~3.8x issue throughput vs 1 queue,
    ~4x measured end to end: 1215us -> 323us per iteration).
    Outstanding gather DMAs are completion-gated at GCAP=6: with >= 7
    in flight alongside other engine traffic, completion semaphores
    fire unreliably on HW and the results are corrupted.
  * ReLU + accumulate is a single fused DVE op per segment:
    acc = max(psum, 0) + acc (scalar_tensor_tensor).
  * dma_gather indices are int16, so the per-core edge stream is cut
    into chunks; each chunk gets a private, deduplicated bank of
    source rows (<= 32768 rows) and locally remapped indices.
    Bank row 0 is all-zeros and used for padding (relu(0@W)=0).
  * Cores are fully independent (no collectives); the host
    concatenates the 8 output shards and undoes the degree-sort
    permutation.
"""

from contextlib import ExitStack

import numpy as np

try:
    import concourse.bass as bass
except ImportError:  # toolchain checkout not on sys.path
    import sys

    sys.path.insert(0, "/opt/trn_rl_repo")
    import concourse.bass as bass

import ml_dtypes

import concourse.bacc as bacc
import concourse.mybir as mybir
from concourse import library_config
from concourse.bass_utils import run_bass_kernel_spmd

BF16 = mybir.dt.bfloat16
F32 = mybir.dt.float32
I16 = mybir.dt.int16

N_NODES = 100000
N_EDGES = 800000
D = 128
CORES = 8
NPC = N_NODES // CORES  # nodes per core

GT = 512  # gather tile (edges per dma_gather call); multiple of 128, <= 512
# (the gather ucode hard-crashes the device for num_idxs > 512)
N_QUEUES = 4  # SWDGE queues used for gathers (1-4)
MM_N = 512  # max matmul free dim / PSUM bank width (fp32)
NB = 8  # PSUM banks used (max 8)
BUFS = 16  # gather staging buffers (deep pipeline across 4 queues)
GCAP = 6  # max outstanding gather DMAs (completion-gated on the Pool engine)
IDX_CAP = 32767  # max int16 index (bank row); row 0 reserved for zeros


# --------------------------------------------------------------------------
# Host-side planning
# --------------------------------------------------------------------------
class Plan:
    pass


def _build_plan(src, dst):
    src = np.asarray(src).astype(np.int64)
    dst = np.asarray(dst).astype(np.int64)

    owner = dst // NPC
    per_core = []
    for c in range(CORES):
        sel = np.nonzero(owner == c)[0]
        ldst = dst[sel] - c * NPC
        lsrc = src[sel]
        deg = np.bincount(ldst, minlength=NPC)
        perm = np.argsort(-deg, kind="stable")  # node id for each slot
        deg_sorted = deg[perm]
        slot = np.empty(NPC, np.int64)
        slot[perm] = np.arange(NPC)
        order = np.argsort(slot[ldst], kind="stable")
        src_sorted = lsrc[order]
        run_start = np.zeros(NPC, np.int64)
        run_start[1:] = np.cumsum(deg_sorted)[:-1]
        per_core.append(
            dict(
                perm=perm,
                deg_sorted=deg_sorted,
                src_sorted=src_sorted,
                run_start=run_start,
            )
        )

    maxdeg = int(max(int(pc["deg_sorted"][0]) for pc in per_core))
    # padded per-round widths, shared by all cores (SPMD: one program)
    pcnt = []
    for k in range(maxdeg):
        cnt = max(int((pc["deg_sorted"] > k).sum()) for pc in per_core)
        pcnt.append(-(-cnt // 128) * 128)
    round_start = np.zeros(maxdeg + 1, np.int64)
    round_start[1:] = np.cumsum(pcnt)
    L = int(round_start[-1])
    L_pad = -(-L // GT) * GT

    # flat gather value stream per core (-1 = padding)
    gather_vals = np.full((CORES, L_pad), -1, np.int64)
    for c, pc in enumerate(per_core):
        ds_, ss, rs = pc["deg_sorted"], pc["src_sorted"], pc["run_start"]
        for k in range(maxdeg):
            cnt_k = int((ds_ > k).sum())
            if cnt_k:
                o = int(round_start[k])
                gather_vals[c, o : o + cnt_k] = ss[rs[:cnt_k] + k]

    # tiles and matmul segments; new_round marks segments whose accumulator
    # columns may overlap earlier segments' (needs a DVE pipeline drain)
    n_tiles = L_pad // GT
    tiles = []  # per tile: list of (local_off, width, acc_col, new_round)
    for t in range(n_tiles):
        a, b = t * GT, (t + 1) * GT
        segs = []
        for k in range(maxdeg):
            rs, re = int(round_start[k]), int(round_start[k + 1])
            lo, hi = max(a, rs), min(b, re)
            o = lo
            while o < hi:
                w = min(MM_N, hi - o)
                segs.append((o - a, w, o - rs, k > 0 and o == rs))
                o += w
        tiles.append(segs)

    # greedy chunking of tiles under the int16 index cap
    chunks = []  # list of (tile_start, tile_end)
    cs = 0
    while cs < n_tiles:
        ce = cs + 1
        while ce < n_tiles:
            ok = True
            for c in range(CORES):
                v = gather_vals[c, cs * GT : (ce + 1) * GT]
                if len(np.unique(v[v >= 0])) + 1 > IDX_CAP:
                    ok = False
                    break
            if not ok:
                break
            ce += 1
        chunks.append((cs, ce))
        cs = ce

    # per-chunk banks + remapped int16 indices
    idx16 = np.zeros((CORES, L_pad), np.int16)
    bank_uniqs = []  # per chunk: list per core of unique src node ids
    bank_rows = []
    for j, (cs, ce) in enumerate(chunks):
        a, b = cs * GT, ce * GT
        uniqs = []
        rows = 0
        for c in range(CORES):
            v = gather_vals[c, a:b]
            valid = v >= 0
            u = np.unique(v[valid])
            assert len(u) + 1 <= IDX_CAP + 1
            loc = np.zeros(b - a, np.int16)
            loc[valid] = (np.searchsorted(u, v[valid]) + 1).astype(np.int16)
            idx16[c, a:b] = loc
            uniqs.append(u)
            rows = max(rows, len(u) + 1)
        bank_uniqs.append(uniqs)
        bank_rows.append(-(-rows // 128) * 128)

    p = Plan()
    p.per_core = per_core
    p.maxdeg = maxdeg
    p.L_pad = L_pad
    p.n_tiles = n_tiles
    p.tiles = tiles
    p.chunks = chunks
    p.chunk_of_tile = np.zeros(n_tiles, np.int64)
    for j, (cs, ce) in enumerate(chunks):
        p.chunk_of_tile[cs:ce] = j
    p.idx16 = idx16
    p.bank_uniqs = bank_uniqs
    p.bank_rows = bank_rows
    p.acc_cols = max(pcnt) if pcnt else 128
    p.n_segs = sum(len(s) for s in tiles)
    return p


def _build_in_maps(plan, h, W):
    h = np.asarray(h, np.float32)
    W = np.asarray(W, np.float32)
    h_hi = h.astype(ml_dtypes.bfloat16)
    h_lo = (h - h_hi.astype(np.float32)).astype(ml_dtypes.bfloat16)
    Wt = np.ascontiguousarray(W.T)  # [in, out]
    wt_hi = Wt.astype(ml_dtypes.bfloat16)
    wt_lo = (Wt - wt_hi.astype(np.float32)).astype(ml_dtypes.bfloat16)

    in_maps = []
    for c in range(CORES):
        m = {"whi": wt_hi, "wlo": wt_lo}
        # idx stream: [128, L/16] int16; position i lives at [i%16, i//16],
        # replicated across the 8 groups of 16 partitions
        flat = plan.idx16[c]
        arr16 = flat.reshape(-1, 16).T  # [16, L/16]
        m["idx"] = np.ascontiguousarray(np.tile(arr16, (8, 1)))
        for j, (_cs, _ce) in enumerate(plan.chunks):
            u = plan.bank_uniqs[j][c]
            bank = np.zeros((plan.bank_rows[j], 2 * D), ml_dtypes.bfloat16)
            bank[1 : 1 + len(u), :D] = h_hi[u]
            bank[1 : 1 + len(u), D:] = h_lo[u]
            m[f"bank{j}"] = bank
        in_maps.append(m)
    return in_maps


# --------------------------------------------------------------------------
# Device program (raw bass, SPMD: same program on all cores)
# --------------------------------------------------------------------------
def _build_nc(plan, reps=1, loop_n=None):
    # reps>1 concatenates the whole edge stream `reps` times (same data) so
    # per-iteration HW time can be measured as (T(reps)-T(1))/(reps-1);
    # the output is then reps*correct, which only timing runs use.
    # loop_n wraps the per-iteration pipeline in a device-side Fori with a
    # 3-phase all-engine barrier + semaphore reset at the back edge, so
    # thousands of iterations fit in one NEFF (timing only; the CoreSim race
    # detector doesn't understand hand-rolled barrier resets, so it's off).
    nc = bacc.Bacc(
        "TRN2",
        detect_race_conditions=(loop_n is None),
        num_swdge_queues=N_QUEUES,
    )
    L = plan.L_pad

    whi_d = nc.dram_tensor("whi", [D, D], BF16, kind="ExternalInput")
    wlo_d = nc.dram_tensor("wlo", [D, D], BF16, kind="ExternalInput")
    idx_d = nc.dram_tensor("idx", [128, L // 16], I16, kind="ExternalInput")
    banks_d = [
        nc.dram_tensor(f"bank{j}", [plan.bank_rows[j], 2 * D], BF16,
                       kind="ExternalInput")
        for j in range(len(plan.chunks))
    ]
    out_d = nc.dram_tensor("out", [D, NPC], F32, kind="ExternalOutput")

    n_tiles = plan.n_tiles
    tiles = plan.tiles
    n_segs = plan.n_segs
    # global segment index of the first segment of each tile
    seg_base = np.zeros(n_tiles + 1, np.int64)
    for t in range(n_tiles):
        seg_base[t + 1] = seg_base[t] + len(tiles[t])

    with (
        nc.sbuf_tensor("whi_s", [D, D], BF16) as whi_s,
        nc.sbuf_tensor("wlo_s", [D, D], BF16) as wlo_s,
        nc.sbuf_tensor("idx_s", [128, L // 16], I16) as idx_s,
        nc.sbuf_tensor("acc", [128, plan.acc_cols], F32) as acc,
        nc.sbuf_tensor("gbuf", [128, BUFS, 2, GT], BF16) as gbuf,
        nc.psum_tensor("ps", [128, NB, MM_N], F32) as ps,
        nc.semaphore("io_sem") as io_sem,
        nc.semaphore("mm_sem") as mm_sem,
        nc.semaphore("dve_sem") as dve_sem,
        nc.semaphore("init_sem") as init_sem,
        ExitStack() as _sems,
        nc.Block() as block,
    ):
        gat_sems = [
            _sems.enter_context(nc.semaphore(f"gat_sem{i}")) for i in range(BUFS)
        ]
        bars = [_sems.enter_context(nc.semaphore(f"bar{i}")) for i in range(3)]
        # per-iteration final value of each work semaphore (clear-safety waits)
        work_finals = [
            (gs, 16 * len([t for t in range(n_tiles) if t % BUFS == i]))
            for i, gs in enumerate(gat_sems)
        ] + [(mm_sem, n_segs), (dve_sem, n_segs)]

        def barrier(eng, is_sync):
            # 3-phase all-engine barrier; sync resets the work semaphores so
            # every loop iteration reuses the same wait immediates.  Each
            # clear happens while every other engine is provably blocked
            # before its next inc of that semaphore: a sem cleared between
            # barrier k and sync's barrier-k inc can only be inc'd again
            # after the peers pass a *later* barrier that sync's inc gates.
            eng.sem_inc(bars[0], 1)
            eng.wait_ge(bars[0], 4)
            if is_sync:
                for s_, fin in work_finals:
                    if fin:
                        eng.wait_ge(s_, fin)
                    eng.sem_clear(s_)
                eng.sem_clear(bars[2])
            eng.sem_inc(bars[1], 1)
            eng.wait_ge(bars[1], 4)
            if is_sync:
                eng.sem_clear(bars[0])
            eng.sem_inc(bars[2], 1)
            eng.wait_ge(bars[2], 4)
            if is_sync:
                eng.sem_clear(bars[1])

        def pool_iter(g, gt_reg, rep):
            for t in range(n_tiles):
                tt = rep * n_tiles + t
                if tt >= GCAP:
                    tc_ = tt - GCAP
                    g.wait_ge(gat_sems[tc_ % BUFS], 16 * (tc_ // BUFS + 1))
                if tt >= BUFS and loop_n is None:
                    ttb = tt - BUFS + 1
                    base = (ttb // n_tiles) * n_segs + int(seg_base[ttb % n_tiles])
                    g.wait_ge(mm_sem, base)
                elif loop_n is not None and t >= BUFS:
                    g.wait_ge(mm_sem, int(seg_base[t - BUFS + 1]))
                g.dma_gather(
                    gbuf[:, tt % BUFS, :, :],
                    banks_d[int(plan.chunk_of_tile[t])][:, :],
                    idx_s[:, t * (GT // 16) : (t + 1) * (GT // 16)],
                    num_idxs=GT,
                    num_idxs_reg=gt_reg,
                    elem_size=2 * D,
                    transpose=True,
                    single_packet=True,
                    queue_num=t % N_QUEUES,
                ).then_inc(gat_sems[tt % BUFS], 16)

        def pe_iter(te, rep):
            s = rep * n_segs
            for t in range(n_tiles):
                tt = rep * n_tiles + t
                te.wait_ge(gat_sems[tt % BUFS], 16 * (tt // BUFS + 1))
                for off, w, _col, _nr in tiles[t]:
                    b = s % NB
                    if s >= NB:
                        te.wait_ge(dve_sem, s - NB + 1)
                    pw = ps[:, b, :w]
                    g0 = gbuf[:, tt % BUFS, 0, off : off + w]
                    g1 = gbuf[:, tt % BUFS, 1, off : off + w]
                    te.matmul(pw, whi_s[:, :], g0, start=True, stop=False)
                    te.matmul(pw, wlo_s[:, :], g0, start=False, stop=False)
                    te.matmul(pw, whi_s[:, :], g1, start=False, stop=True).then_inc(
                        mm_sem, 1
                    )
                    s += 1

        def dve_iter(v, rep):
            s = rep * n_segs
            for t in range(n_tiles):
                for _off, w, col, new_round in tiles[t]:
                    v.wait_ge(mm_sem, s + 1)
                    if new_round or (s > 0 and s % n_segs == 0):
                        # prior rounds write overlapping acc columns; DVE has
                        # no same-engine RAW interlock - drain via own sem
                        v.wait_ge(dve_sem, s)
                    v.scalar_tensor_tensor(
                        out=acc[:, col : col + w],
                        in0=ps[:, s % NB, :w],
                        scalar=0.0,
                        in1=acc[:, col : col + w],
                        op0=mybir.AluOpType.max,
                        op1=mybir.AluOpType.add,
                    ).then_inc(dve_sem, 1)
                    s += 1

        @block.sync
        def _(sync):
            sync.dma_start(out=whi_s[:, :], in_=whi_d[:, :]).then_inc(io_sem, 16)
            sync.dma_start(out=wlo_s[:, :], in_=wlo_d[:, :]).then_inc(io_sem, 16)
            sync.dma_start(out=idx_s[:, :], in_=idx_d[:, :]).then_inc(io_sem, 16)
            if loop_n is not None:
                with sync.Fori(0, loop_n):
                    sync.wait_ge(dve_sem, n_segs)
                    barrier(sync, True)
            else:
                sync.wait_ge(dve_sem, n_segs * reps)
            sync.dma_start(out=out_d[:, :], in_=acc[:, :NPC]).then_inc(io_sem, 16)
            sync.wait_ge(io_sem, 64)

        @block.gpsimd
        def _(g):
            g.load_library(library_config.mlp)
            g.wait_ge(io_sem, 48)
            gt_reg = g.to_reg(GT)  # one register, reused by every gather
            if loop_n is not None:
                with g.Fori(0, loop_n):
                    pool_iter(g, gt_reg, 0)
                    barrier(g, False)
            else:
                for rep in range(reps):
                    pool_iter(g, gt_reg, rep)

        @block.tensor
        def _(te):
            te.wait_ge(io_sem, 48)
            if loop_n is not None:
                with te.Fori(0, loop_n):
                    pe_iter(te, 0)
                    barrier(te, False)
            else:
                for rep in range(reps):
                    pe_iter(te, rep)

        @block.vector
        def _(v):
            v.memset(acc[:, :], 0.0).then_inc(init_sem, 1)
            v.wait_ge(init_sem, 1)
            if loop_n is not None:
                with v.Fori(0, loop_n):
                    dve_iter(v, 0)
                    barrier(v, False)
            else:
                for rep in range(reps):
                    dve_iter(v, rep)

    nc.compile()
    return nc


# --------------------------------------------------------------------------
# Entry point
# --------------------------------------------------------------------------
def _assemble(plan, results):
    out = np.empty((N_NODES, D), np.float32)
    for c in range(CORES):
        shard = results[c]["out"]  # [128, NPC], column j = node perm[j]
        out[c * NPC + plan.per_core[c]["perm"]] = shard[:, :NPC].T
    return out


def run(h, W, src, dst, trace=False, reps=1, plan=None):
    if plan is None:
        plan = _build_plan(src, dst)
    nc = _build_nc(plan, reps=reps)
    in_maps = _build_in_maps(plan, h, W)
    res = run_bass_kernel_spmd(nc, in_maps, core_ids=list(range(CORES)), trace=trace)
    return _assemble(plan, res.results), res


def kernel(h, W, src, dst):
    out, _ = run(h, W, src, dst)
    return out

